# revision 5
# baseline (speedup 1.0000x reference)
"""Trainium2 Bass kernel for BlockwiseParallelTransformerAttention scores.

Computes, for full inputs x[2,2048,2048], Wq/Wk/Wv[2048,2048]:
    q = (x @ Wq.T) * 1/sqrt(128), reshaped [B,S,16,128]
    k = (x @ Wk.T),               reshaped [B,S,16,128]
    attn_weights = einsum('bqhd,bkhd->bqhk')   # [2,2048,16,2048]
    attn_output  = zeros([2,2048,2048])
(v is computed-then-discarded by the reference module, so it is skipped.)

Sharding: head-parallel across 8 NeuronCores (2 heads each); Wq/Wk are
column-sharded (rows of the [out,in] weight), x is replicated. Each core
writes its [B,S,2,S] slice of the score tensor; the host concatenates
along the head axis.

On-device layout: all matmuls contract over the partition dim, so x is
pre-transposed on the host to xT[IN, B*S] and weights to W.T[IN, 256],
both cast to bf16 (fp32 accumulation in PSUM). Phase 1 streams xT in
512-token passes, accumulating qT/kT[128d, tokens] per head in PSUM over
the 16 IN-chunks. Phase 2 computes scores q_chunk.T @ kT per (batch,
head) straight from SBUF-resident qT/kT and DMAs [128q, 2*2048] rows out.
"""

import math

import numpy as np
import ml_dtypes

import concourse.bass as bass
import concourse.bacc as bacc
import concourse.mybir as mybir
import concourse.tile as tile
from concourse.bass_utils import run_bass_kernel_spmd

B, S, IN = 2, 2048, 2048
HID, H = 2048, 16
D = 128
NCORES = 8
HL = H // NCORES          # heads per core = 2
HD = HL * D               # per-core hidden slice = 256
TOK = B * S               # 4096 flattened tokens
KC = IN // 128            # 16 contraction chunks
TQ = 512                  # tokens per projection pass
NPASS = TOK // TQ         # 8 passes
SCALE = 1.0 / math.sqrt(D)

F32 = mybir.dt.float32
BF16 = mybir.dt.bfloat16




def _kernel_body(tc, xT, wqT, wkT, out):
    nc = tc.nc
    xT_r = xT.ap().rearrange("(c p) t -> p c t", p=128)    # [128, KC, TOK]
    wq_r = wqT.ap().rearrange("(c p) m -> p c m", p=128)   # [128, KC, HD]
    wk_r = wkT.ap().rearrange("(c p) m -> p c m", p=128)
    out_r = out.ap().rearrange("b (n p) h k -> b n p (h k)", p=128)  # [B,16,128,HL*S]

    with (
        tc.tile_pool(name="w", bufs=1) as wpool,
        tc.tile_pool(name="x", bufs=3) as xpool,
        tc.tile_pool(name="qk", bufs=1) as qkpool,
        tc.tile_pool(name="acc", bufs=1, space="PSUM") as accp,
        tc.tile_pool(name="scp", bufs=4, space="PSUM") as scp,
        tc.tile_pool(name="stage", bufs=4) as stp,
    ):
        wq_sb = wpool.tile([128, KC, HD], BF16, tag="wq")
        nc.sync.dma_start(wq_sb[:, :, :], wq_r)
        wk_sb = wpool.tile([128, KC, HD], BF16, tag="wk")
        nc.sync.dma_start(wk_sb[:, :, :], wk_r)

        # qT/kT[b]: [128 d, HL, S] bf16, SBUF-resident per batch
        qT = [qkpool.tile([128, HL, S], BF16, tag=f"qT{b}", name=f"qT{b}") for b in range(B)]
        kT = [qkpool.tile([128, HL, S], BF16, tag=f"kT{b}", name=f"kT{b}") for b in range(B)]

        def proj_pass(p):
            t0 = p * TQ
            b, off = t0 // S, t0 % S
            xq = xpool.tile([128, KC, TQ], BF16, tag="xq")
            nc.sync.dma_start(xq[:, :, :], xT_r[:, :, t0 : t0 + TQ])
            # (dest-tile, head) per accumulator: q head0, q head1, k head0, k head1
            plan = [(qT[b], wq_sb, 0), (qT[b], wq_sb, 1), (kT[b], wk_sb, 0), (kT[b], wk_sb, 1)]
            accs = [accp.tile([128, TQ], F32, tag=f"acc{j}", name=f"acc{j}_{p}") for j in range(4)]
            for i in range(KC):
                for j, (_, w_sb, h) in enumerate(plan):
                    nc.tensor.matmul(
                        accs[j][:, :],
                        lhsT=w_sb[:, i, h * D : (h + 1) * D],
                        rhs=xq[:, i, :],
                        start=(i == 0),
                        stop=(i == KC - 1),
                    )
            for j, (dest, _, h) in enumerate(plan):
                eng = nc.vector if j % 2 == 0 else nc.scalar
                if eng is nc.vector:
                    eng.tensor_copy(dest[:, h, off : off + TQ], accs[j][:, :])
                else:
                    eng.copy(dest[:, h, off : off + TQ], accs[j][:, :])

        def scores_chunk(b, qcs):
            for qc in qcs:
                stage = stp.tile([128, HL * S], F32, tag="stage", name=f"stage_{b}_{qc}")
                for h in range(HL):
                    lq = qT[b][:, h, qc * 128 : (qc + 1) * 128]
                    for kseg in range(4):  # 512 k-columns per PSUM bank
                        ps = scp.tile([128, 512], F32, tag="sc", name=f"sc_{b}_{qc}_{h}_{kseg}")
                        nc.tensor.matmul(
                            ps[:, :],
                            lhsT=lq,
                            rhs=kT[b][:, h, kseg * 512 : (kseg + 1) * 512],
                            start=True,
                            stop=True,
                        )
                        dst = stage[:, h * S + kseg * 512 : h * S + (kseg + 1) * 512]
                        # 2:1 DVE:ACT split (ACT is ~2x slower per copy)
                        if (2 * h + kseg) % 3 < 2:
                            nc.vector.tensor_copy(dst, ps[:, :])
                        else:
                            nc.scalar.copy(dst, ps[:, :])
                nc.scalar.dma_start(out_r[b, qc], stage[:, :])

        # Batch-0 projections first; then interleave batch-1 projection
        # passes with batch-0 score chunks so score write-DMA overlaps
        # projection compute and x loads never starve behind stores.
        for p in range(NPASS // 2):
            proj_pass(p)
        for g in range(4):
            proj_pass(NPASS // 2 + g)
            scores_chunk(0, range(4 * g, 4 * g + 4))
        scores_chunk(1, range(S // 128))


_NC = None


def _get_nc():
    global _NC
    if _NC is None:
        nc = bacc.Bacc("TRN2", debug=False)
        xT = nc.dram_tensor("xT", [IN, TOK], BF16, kind="ExternalInput")
        wqT = nc.dram_tensor("wqT", [IN, HD], BF16, kind="ExternalInput")
        wkT = nc.dram_tensor("wkT", [IN, HD], BF16, kind="ExternalInput")
        out = nc.dram_tensor("scores", [B, S, HL, S], F32, kind="ExternalOutput")
        with tile.TileContext(nc) as tc:
            _kernel_body(tc, xT, wqT, wkT, out)
        nc.compile()
        _NC = nc
    return _NC


def _prepare(x, Wq, Wk):
    bf = ml_dtypes.bfloat16
    xT = np.ascontiguousarray(x.reshape(TOK, IN).T).astype(bf)
    in_maps = []
    for c in range(NCORES):
        rows = slice(c * HD, (c + 1) * HD)
        in_maps.append({
            "xT": xT,
            "wqT": np.ascontiguousarray((Wq[rows, :] * SCALE).T).astype(bf),
            "wkT": np.ascontiguousarray(Wk[rows, :].T).astype(bf),
        })
    return in_maps


def _run(x, Wq, Wk, trace=False):
    in_maps = _prepare(x, Wq, Wk)
    res = run_bass_kernel_spmd(_get_nc(), in_maps, core_ids=list(range(NCORES)), trace=trace)
    attn_weights = np.concatenate([res.results[c]["scores"] for c in range(NCORES)], axis=2)
    attn_output = np.zeros((B, S, HID), dtype=np.float32)
    return (attn_output, attn_weights), res


def kernel(x, Wq, Wk, Wv):
    x = np.asarray(x, dtype=np.float32)
    Wq = np.asarray(Wq, dtype=np.float32)
    Wk = np.asarray(Wk, dtype=np.float32)
    out, _ = _run(x, Wq, Wk, trace=False)
    return out


def kernel_traced(x, Wq, Wk, Wv):
    out, res = _run(
        np.asarray(x, dtype=np.float32),
        np.asarray(Wq, dtype=np.float32),
        np.asarray(Wk, dtype=np.float32),
        trace=True,
    )
    return out, res


# revision 7
# speedup vs baseline: 1.0491x; 1.0491x over previous
"""Trainium2 Bass kernel for BlockwiseParallelTransformerAttention scores.

Computes, for full inputs x[2,2048,2048], Wq/Wk/Wv[2048,2048]:
    q = (x @ Wq.T) * 1/sqrt(128), reshaped [B,S,16,128]
    k = (x @ Wk.T),               reshaped [B,S,16,128]
    attn_weights = einsum('bqhd,bkhd->bqhk')   # [2,2048,16,2048]
    attn_output  = zeros([2,2048,2048])
(v is computed-then-discarded by the reference module, so it is skipped.)

Sharding: head-parallel across 8 NeuronCores (2 heads each); Wq/Wk are
column-sharded (rows of the [out,in] weight), x is replicated. Each core
writes its [B,S,2,S] slice of the score tensor; the host concatenates
along the head axis.

On-device layout: all matmuls contract over the partition dim, so x is
pre-transposed on the host to xT[IN, B*S] and weights to W.T[IN, 256],
both cast to bf16 (fp32 accumulation in PSUM). Phase 1 streams xT in
512-token passes, accumulating qT/kT[128d, tokens] per head in PSUM over
the 16 IN-chunks. Phase 2 computes scores q_chunk.T @ kT per (batch,
head) straight from SBUF-resident qT/kT and DMAs [128q, 2*2048] rows out.
"""

import math

import numpy as np
import ml_dtypes

import concourse.bass as bass
import concourse.bacc as bacc
import concourse.mybir as mybir
import concourse.tile as tile
from concourse.bass_utils import run_bass_kernel_spmd

B, S, IN = 2, 2048, 2048
HID, H = 2048, 16
D = 128
NCORES = 8
HL = H // NCORES          # heads per core = 2
HD = HL * D               # per-core hidden slice = 256
TOK = B * S               # 4096 flattened tokens
KC = IN // 128            # 16 contraction chunks
TQ = 512                  # tokens per projection pass
NPASS = TOK // TQ         # 8 passes
SCALE = 1.0 / math.sqrt(D)

F32 = mybir.dt.float32
BF16 = mybir.dt.bfloat16




def _kernel_body(tc, xT, wqT, wkT, out):
    nc = tc.nc
    xT_r = xT.ap().rearrange("(c p) t -> p c t", p=128)    # [128, KC, TOK]
    wq_r = wqT.ap().rearrange("(c p) m -> p c m", p=128)   # [128, KC, HD]
    wk_r = wkT.ap().rearrange("(c p) m -> p c m", p=128)
    out_r = out.ap().rearrange("b (n p) h k -> b n p (h k)", p=128)  # [B,16,128,HL*S]

    TH = 1024            # tokens per projection pass (half a batch)
    NSEG = TH // 512     # 512-wide matmul segments per pass

    with (
        tc.tile_pool(name="w", bufs=1) as wpool,
        tc.tile_pool(name="x", bufs=3) as xpool,
        tc.tile_pool(name="qk", bufs=1) as qkpool,
        tc.tile_pool(name="acc", bufs=1, space="PSUM") as accp,
        tc.tile_pool(name="scp", bufs=4, space="PSUM") as scp,
        tc.tile_pool(name="stage", bufs=4) as stp,
    ):
        wq_sb = wpool.tile([128, KC, HD], BF16, tag="wq")
        nc.sync.dma_start(wq_sb[:, :, :], wq_r)
        wk_sb = wpool.tile([128, KC, HD], BF16, tag="wk")
        nc.sync.dma_start(wk_sb[:, :, :], wk_r)

        # qT/kT[b]: [128 d, HL, S] bf16, SBUF-resident per batch
        qT = [qkpool.tile([128, HL, S], BF16, tag=f"qT{b}", name=f"qT{b}") for b in range(B)]
        kT = [qkpool.tile([128, HL, S], BF16, tag=f"kT{b}", name=f"kT{b}") for b in range(B)]

        xh_tiles = {}

        def get_xh(b, half):
            key = (b, half)
            if key not in xh_tiles:
                t0 = b * S + half * TH
                xh = xpool.tile([128, KC, TH], BF16, tag="xh", name=f"xh_{b}_{half}")
                # split the 4MB load so the first matmul starts ~4x sooner
                for g in range(4):
                    nc.sync.dma_start(
                        xh[:, 4 * g : 4 * g + 4, :],
                        xT_r[:, 4 * g : 4 * g + 4, t0 : t0 + TH],
                    )
                xh_tiles[key] = xh
            return xh_tiles[key]

        pass_idx = [0]

        def proj_pass(b, h, half, w_sb, dest):
            xh = get_xh(b, half)
            n = pass_idx[0]
            pass_idx[0] += 1
            pp = accp.tile([128, TH], F32, tag=f"acc{n % 2}", name=f"pp{n}")
            for i in range(KC):
                for s in range(NSEG):
                    nc.tensor.matmul(
                        pp[:, s * 512 : (s + 1) * 512],
                        lhsT=w_sb[:, i, h * D : (h + 1) * D],
                        rhs=xh[:, i, s * 512 : (s + 1) * 512],
                        start=(i == 0),
                        stop=(i == KC - 1),
                    )
            dst = dest[:, h, half * TH : (half + 1) * TH]
            if n % 2 == 0:
                nc.vector.tensor_copy(dst, pp[:, :])
            else:
                nc.scalar.copy(dst, pp[:, :])

        def scores_head(b, h):
            for qc in range(S // 128):
                stage = stp.tile([128, S], F32, tag="stage", name=f"stage_{b}_{h}_{qc}")
                lq = qT[b][:, h, qc * 128 : (qc + 1) * 128]
                for kseg in range(4):
                    ps = scp.tile([128, 512], F32, tag="sc", name=f"sc_{b}_{h}_{qc}_{kseg}")
                    nc.tensor.matmul(
                        ps[:, :],
                        lhsT=lq,
                        rhs=kT[b][:, h, kseg * 512 : (kseg + 1) * 512],
                        start=True,
                        stop=True,
                    )
                    dst = stage[:, kseg * 512 : (kseg + 1) * 512]
                    if kseg % 2 == 0:
                        nc.vector.tensor_copy(dst, ps[:, :])
                    else:
                        nc.scalar.copy(dst, ps[:, :])
                nc.scalar.dma_start(out_r[b, qc][:, h * S : (h + 1) * S], stage[:, :])

        # Per (batch, head): project k then q for both token halves, then
        # immediately emit that head's scores — score writes start flowing
        # after the first head's projections (~25% into the kernel) and
        # overlap all remaining projection work.
        for b in range(B):
            for h in range(HL):
                for half in range(2):
                    proj_pass(b, h, half, wk_sb, kT[b])
                    proj_pass(b, h, half, wq_sb, qT[b])
                scores_head(b, h)


_NC = None


def _get_nc():
    global _NC
    if _NC is None:
        nc = bacc.Bacc("TRN2", debug=False)
        xT = nc.dram_tensor("xT", [IN, TOK], BF16, kind="ExternalInput")
        wqT = nc.dram_tensor("wqT", [IN, HD], BF16, kind="ExternalInput")
        wkT = nc.dram_tensor("wkT", [IN, HD], BF16, kind="ExternalInput")
        out = nc.dram_tensor("scores", [B, S, HL, S], F32, kind="ExternalOutput")
        with tile.TileContext(nc) as tc:
            _kernel_body(tc, xT, wqT, wkT, out)
        nc.compile()
        _NC = nc
    return _NC


def _prepare(x, Wq, Wk):
    bf = ml_dtypes.bfloat16
    xT = np.ascontiguousarray(x.reshape(TOK, IN).T).astype(bf)
    in_maps = []
    for c in range(NCORES):
        rows = slice(c * HD, (c + 1) * HD)
        in_maps.append({
            "xT": xT,
            "wqT": np.ascontiguousarray((Wq[rows, :] * SCALE).T).astype(bf),
            "wkT": np.ascontiguousarray(Wk[rows, :].T).astype(bf),
        })
    return in_maps


def _run(x, Wq, Wk, trace=False):
    in_maps = _prepare(x, Wq, Wk)
    res = run_bass_kernel_spmd(_get_nc(), in_maps, core_ids=list(range(NCORES)), trace=trace)
    attn_weights = np.concatenate([res.results[c]["scores"] for c in range(NCORES)], axis=2)
    attn_output = np.zeros((B, S, HID), dtype=np.float32)
    return (attn_output, attn_weights), res


def kernel(x, Wq, Wk, Wv):
    x = np.asarray(x, dtype=np.float32)
    Wq = np.asarray(Wq, dtype=np.float32)
    Wk = np.asarray(Wk, dtype=np.float32)
    out, _ = _run(x, Wq, Wk, trace=False)
    return out


def kernel_traced(x, Wq, Wk, Wv):
    out, res = _run(
        np.asarray(x, dtype=np.float32),
        np.asarray(Wq, dtype=np.float32),
        np.asarray(Wk, dtype=np.float32),
        trace=True,
    )
    return out, res


# revision 17
# speedup vs baseline: 1.0723x; 1.0221x over previous
"""Trainium2 Bass kernel for BlockwiseParallelTransformerAttention scores.

Computes, for full inputs x[2,2048,2048], Wq/Wk/Wv[2048,2048]:
    q = (x @ Wq.T) * 1/sqrt(128), reshaped [B,S,16,128]
    k = (x @ Wk.T),               reshaped [B,S,16,128]
    attn_weights = einsum('bqhd,bkhd->bqhk')   # [2,2048,16,2048]
    attn_output  = zeros([2,2048,2048])
(v is computed-then-discarded by the reference module, so it is skipped.)

Sharding: head-parallel across 8 NeuronCores (2 heads each); Wq/Wk are
column-sharded (rows of the [out,in] weight), x is replicated. Each core
writes its [B,S,2,S] slice of the score tensor; the host concatenates
along the head axis.

On-device layout: all matmuls contract over the partition dim, so x is
pre-transposed on the host to xT[IN, B*S] and weights to W.T[IN, 256],
both cast to bf16 (fp32 accumulation in PSUM). Phase 1 streams xT in
512-token passes, accumulating qT/kT[128d, tokens] per head in PSUM over
the 16 IN-chunks. Phase 2 computes scores q_chunk.T @ kT per (batch,
head) straight from SBUF-resident qT/kT and DMAs [128q, 2*2048] rows out.
"""

import math

import numpy as np
import ml_dtypes

import concourse.bass as bass
import concourse.bacc as bacc
import concourse.bass_utils as _bu

import concourse.bacc as bacc
import concourse.mybir as mybir
import concourse.tile as tile
from concourse.bass_utils import run_bass_kernel_spmd

B, S, IN = 2, 2048, 2048
HID, H = 2048, 16
D = 128
NCORES = 8
HL = H // NCORES          # heads per core = 2
HD = HL * D               # per-core hidden slice = 256
TOK = B * S               # 4096 flattened tokens
KC = IN // 128            # 16 contraction chunks
TQ = 512                  # tokens per projection pass
NPASS = TOK // TQ         # 8 passes
SCALE = 1.0 / math.sqrt(D)

F32 = mybir.dt.float32
BF16 = mybir.dt.bfloat16




def _kernel_body(tc, xT, wqT, wkT, out):
    nc = tc.nc
    xT_r = xT.ap().rearrange("(c p) t -> p c t", p=128)    # [128, KC, TOK]
    wq_r = wqT.ap().rearrange("(c p) m -> p c m", p=128)   # [128, KC, HD]
    wk_r = wkT.ap().rearrange("(c p) m -> p c m", p=128)
    out_r = out.ap().rearrange("b (n p) h k -> b n p (h k)", p=128)  # [B,16,128,HL*S]

    TH = 1024            # tokens per projection pass (half a batch)
    NSEG = TH // 512     # 512-wide matmul segments per pass

    with (
        tc.tile_pool(name="w", bufs=1) as wpool,
        tc.tile_pool(name="x", bufs=3) as xpool,
        tc.tile_pool(name="qk", bufs=1) as qkpool,
        tc.tile_pool(name="acc", bufs=1, space="PSUM") as accp,
        tc.tile_pool(name="scp", bufs=2, space="PSUM") as scp,
        tc.tile_pool(name="stage", bufs=4) as stp,
    ):
        wq_sb = wpool.tile([128, KC, HD], BF16, tag="wq")
        wk_sb = wpool.tile([128, KC, HD], BF16, tag="wk")
        for g in range(4):
            cs = slice(4 * g, 4 * g + 4)
            nc.sync.dma_start(wk_sb[:, cs, :], wk_r[:, cs, :])
            nc.sync.dma_start(wq_sb[:, cs, :], wq_r[:, cs, :])

        # qT/kT[b]: [128 d, HL, S] bf16, SBUF-resident per batch
        qT = [qkpool.tile([128, HL, S], BF16, tag=f"qT{b}", name=f"qT{b}") for b in range(B)]
        kT = [qkpool.tile([128, HL, S], BF16, tag=f"kT{b}", name=f"kT{b}") for b in range(B)]

        xh_tiles = {}

        def get_xh(b, half):
            key = (b, half)
            if key not in xh_tiles:
                t0 = b * S + half * TH
                xh = xpool.tile([128, KC, TH], BF16, tag="xh", name=f"xh_{b}_{half}")
                # split the 4MB load so the first matmul starts ~4x sooner
                for g in range(4):
                    nc.sync.dma_start(
                        xh[:, 4 * g : 4 * g + 4, :],
                        xT_r[:, 4 * g : 4 * g + 4, t0 : t0 + TH],
                    )
                xh_tiles[key] = xh
            return xh_tiles[key]

        pass_idx = [0]

        def proj_pass(b, h, half, w_sb, dest):
            xh = get_xh(b, half)
            n = pass_idx[0]
            pass_idx[0] += 1
            pp = accp.tile([128, TH], F32, tag=f"acc{n % 2}", name=f"pp{n}")
            for i in range(KC):
                for s in range(NSEG):
                    nc.tensor.matmul(
                        pp[:, s * 512 : (s + 1) * 512],
                        lhsT=w_sb[:, i, h * D : (h + 1) * D],
                        rhs=xh[:, i, s * 512 : (s + 1) * 512],
                        start=(i == 0),
                        stop=(i == KC - 1),
                    )
            dst = dest[:, h, half * TH : (half + 1) * TH]
            if n % 2 == 0:
                nc.vector.tensor_copy(dst, pp[:, :])
            else:
                nc.scalar.copy(dst, pp[:, :])

        def proj_quarter(b, h, half, seg, w_sb, dest):
            xh = get_xh(b, half)
            n = pass_idx[0]
            pass_idx[0] += 1
            pp = accp.tile([128, 512], F32, tag=f"acc{n % 2}", name=f"ppq{n}")
            for i in range(KC):
                nc.tensor.matmul(
                    pp[:, :],
                    lhsT=w_sb[:, i, h * D : (h + 1) * D],
                    rhs=xh[:, i, seg * 512 : (seg + 1) * 512],
                    start=(i == 0),
                    stop=(i == KC - 1),
                )
            dst = dest[:, h, half * TH + seg * 512 : half * TH + (seg + 1) * 512]
            if n % 2 == 0:
                nc.vector.tensor_copy(dst, pp[:, :])
            else:
                nc.scalar.copy(dst, pp[:, :])

        def score_chunk(b, h, qc):
            stage = stp.tile([128, S], F32, tag="stage", name=f"stage_{b}_{h}_{qc}")
            lq = qT[b][:, h, qc * 128 : (qc + 1) * 128]
            for half in range(2):
                ps = scp.tile([128, 1024], F32, tag="sc", name=f"sc_{b}_{h}_{qc}_{half}")
                for s2 in range(2):
                    nc.tensor.matmul(
                        ps[:, s2 * 512 : (s2 + 1) * 512],
                        lhsT=lq,
                        rhs=kT[b][:, h, half * 1024 + s2 * 512 : half * 1024 + (s2 + 1) * 512],
                        start=True,
                        stop=True,
                    )
                dst = stage[:, half * 1024 : (half + 1) * 1024]
                if (qc + half) % 2 == 0:
                    nc.vector.tensor_copy(dst, ps[:, :])
                else:
                    nc.scalar.copy(dst, ps[:, :])
            nc.scalar.dma_start(out_r[b, qc][:, h * S : (h + 1) * S], stage[:, :])

        # Per (batch, head): project k then q for both token halves; the
        # previous head's score chunks are woven between projection passes
        # so the PE stream alternates (keeps HAM warm, keeps write-DMA fed).
        from collections import deque

        pending = deque()
        for b in range(B):
            for h in range(HL):
                last = (b == B - 1) and (h == HL - 1)
                if not last:
                    for half in range(2):
                        proj_pass(b, h, half, wk_sb, kT[b])
                        for _ in range(min(2, len(pending))):
                            score_chunk(*pending.popleft())
                        proj_pass(b, h, half, wq_sb, qT[b])
                        for _ in range(min(2, len(pending))):
                            score_chunk(*pending.popleft())
                    pending.extend((b, h, qc) for qc in range(S // 128))
                else:
                    # final head: quarter-granular q-projections with its own
                    # score chunks woven immediately behind each quarter, so
                    # the kernel tail never runs scores alone
                    for half in range(2):
                        proj_pass(b, h, half, wk_sb, kT[b])
                        for _ in range(min(2, len(pending))):
                            score_chunk(*pending.popleft())
                    for half in range(2):
                        for seg in range(2):
                            proj_quarter(b, h, half, seg, wq_sb, qT[b])
                            for _ in range(min(2, len(pending))):
                                score_chunk(*pending.popleft())
                            for qc in range(4 * (2 * half + seg), 4 * (2 * half + seg) + 4):
                                score_chunk(b, h, qc)
        while pending:
            score_chunk(*pending.popleft())


def _dedup_ldweights(nc):
    """Drop LDWEIGHTS that reload the identical stationary operand.

    tile_legalize splits every matmul into LDWEIGHTS+MATMUL; the PE array
    keeps the stationary operand across matmuls, so a reload of the same
    AP with no intervening PE weight change is dead. Only sync-free
    instances are removed.
    """
    removed = 0
    for f in nc.m.functions:
        for bb in f.blocks:
            keep = []
            key = None
            for inst in bb.instructions:
                tn = type(inst).__name__
                if getattr(inst, "engine", None) == mybir.EngineType.PE:
                    if tn == "InstLdweights":
                        k = repr(inst.ins[0])
                        si = inst.sync_info
                        clean = not si or (not si.on_wait and not si.on_update)
                        if k == key and clean:
                            removed += 1
                            continue
                        key = k
                    elif tn not in ("InstMatmult", "InstNoOp", "InstEventSemaphore"):
                        key = None
                keep.append(inst)
            bb.instructions[:] = keep
    return removed


_NC = None


def _get_nc():
    global _NC
    if _NC is None:
        nc = bacc.Bacc("TRN2", debug=False)
        xT = nc.dram_tensor("xT", [IN, TOK], BF16, kind="ExternalInput")
        wqT = nc.dram_tensor("wqT", [IN, HD], BF16, kind="ExternalInput")
        wkT = nc.dram_tensor("wkT", [IN, HD], BF16, kind="ExternalInput")
        out = nc.dram_tensor("scores", [B, S, HL, S], F32, kind="ExternalOutput")
        with tile.TileContext(nc) as tc:
            _kernel_body(tc, xT, wqT, wkT, out)
        nc.compile()
        _dedup_ldweights(nc)
        _NC = nc
    return _NC


def _prepare(x, Wq, Wk):
    bf = ml_dtypes.bfloat16
    xT = np.ascontiguousarray(x.reshape(TOK, IN).T).astype(bf)
    in_maps = []
    for c in range(NCORES):
        rows = slice(c * HD, (c + 1) * HD)
        in_maps.append({
            "xT": xT,
            "wqT": np.ascontiguousarray((Wq[rows, :] * SCALE).T).astype(bf),
            "wkT": np.ascontiguousarray(Wk[rows, :].T).astype(bf),
        })
    return in_maps


def _run(x, Wq, Wk, trace=False):
    in_maps = _prepare(x, Wq, Wk)
    res = run_bass_kernel_spmd(_get_nc(), in_maps, core_ids=list(range(NCORES)), trace=trace)
    attn_weights = np.concatenate([res.results[c]["scores"] for c in range(NCORES)], axis=2)
    attn_output = np.zeros((B, S, HID), dtype=np.float32)
    return (attn_output, attn_weights), res


def kernel(x, Wq, Wk, Wv):
    x = np.asarray(x, dtype=np.float32)
    Wq = np.asarray(Wq, dtype=np.float32)
    Wk = np.asarray(Wk, dtype=np.float32)
    out, _ = _run(x, Wq, Wk, trace=False)
    return out


def kernel_traced(x, Wq, Wk, Wv):
    out, res = _run(
        np.asarray(x, dtype=np.float32),
        np.asarray(Wq, dtype=np.float32),
        np.asarray(Wk, dtype=np.float32),
        trace=True,
    )
    return out, res


# revision 19
# speedup vs baseline: 1.1005x; 1.0264x over previous
"""Trainium2 Bass kernel for BlockwiseParallelTransformerAttention scores.

Computes, for full inputs x[2,2048,2048], Wq/Wk/Wv[2048,2048]:
    q = (x @ Wq.T) * 1/sqrt(128), reshaped [B,S,16,128]
    k = (x @ Wk.T),               reshaped [B,S,16,128]
    attn_weights = einsum('bqhd,bkhd->bqhk')   # [2,2048,16,2048]
    attn_output  = zeros([2,2048,2048])
(v is computed-then-discarded by the reference module, so it is skipped.)

Sharding: head-parallel across 8 NeuronCores (2 heads each); Wq/Wk are
column-sharded (rows of the [out,in] weight), x is replicated. Each core
writes its [B,S,2,S] slice of the score tensor; the host concatenates
along the head axis.

On-device layout: all matmuls contract over the partition dim, so x is
pre-transposed on the host to xT[IN, B*S] and weights to W.T[IN, 256],
both cast to bf16 (fp32 accumulation in PSUM). Phase 1 streams xT in
512-token passes, accumulating qT/kT[128d, tokens] per head in PSUM over
the 16 IN-chunks. Phase 2 computes scores q_chunk.T @ kT per (batch,
head) straight from SBUF-resident qT/kT and DMAs [128q, 2*2048] rows out.
"""

import math

import numpy as np
import ml_dtypes

import concourse.bacc as bacc
import concourse.mybir as mybir
import concourse.tile as tile
from concourse.bass_utils import run_bass_kernel_spmd

B, S, IN = 2, 2048, 2048
HID, H = 2048, 16
D = 128
NCORES = 8
HL = H // NCORES          # heads per core = 2
HD = HL * D               # per-core hidden slice = 256
TOK = B * S               # 4096 flattened tokens
KC = IN // 128            # 16 contraction chunks
TQ = 512                  # tokens per projection pass
NPASS = TOK // TQ         # 8 passes
SCALE = 1.0 / math.sqrt(D)

F32 = mybir.dt.float32
BF16 = mybir.dt.bfloat16




def _kernel_body(tc, xT, wqT, wkT, out):
    nc = tc.nc
    xT_r = xT.ap().rearrange("(c p) t -> p c t", p=128)    # [128, KC, TOK]
    wq_r = wqT.ap().rearrange("(c p) m -> p c m", p=128)   # [128, KC, HD]
    wk_r = wkT.ap().rearrange("(c p) m -> p c m", p=128)
    out_r = out.ap().rearrange("b (n p) h k -> b n p (h k)", p=128)  # [B,16,128,HL*S]

    TH = 1024            # tokens per projection pass (half a batch)
    NSEG = TH // 512     # 512-wide matmul segments per pass

    with (
        tc.tile_pool(name="w", bufs=1) as wpool,
        tc.tile_pool(name="x", bufs=3) as xpool,
        tc.tile_pool(name="qk", bufs=1) as qkpool,
        tc.tile_pool(name="acc", bufs=1, space="PSUM") as accp,
        tc.tile_pool(name="scp", bufs=2, space="PSUM") as scp,
        tc.tile_pool(name="stage", bufs=4) as stp,
    ):
        wq_sb = wpool.tile([128, KC, HD], BF16, tag="wq")
        wk_sb = wpool.tile([128, KC, HD], BF16, tag="wk")
        for g in range(4):
            cs = slice(4 * g, 4 * g + 4)
            nc.sync.dma_start(wk_sb[:, cs, :], wk_r[:, cs, :])
            nc.sync.dma_start(wq_sb[:, cs, :], wq_r[:, cs, :])

        # qT/kT[b]: [128 d, HL, S] bf16, SBUF-resident per batch
        qT = [qkpool.tile([128, HL, S], BF16, tag=f"qT{b}", name=f"qT{b}") for b in range(B)]
        kT = [qkpool.tile([128, HL, S], BF16, tag=f"kT{b}", name=f"kT{b}") for b in range(B)]

        xh_tiles = {}

        def get_xh(b, half):
            key = (b, half)
            if key not in xh_tiles:
                t0 = b * S + half * TH
                xh = xpool.tile([128, KC, TH], BF16, tag="xh", name=f"xh_{b}_{half}")
                # split the 4MB load so the first matmul starts ~4x sooner
                for g in range(4):
                    nc.sync.dma_start(
                        xh[:, 4 * g : 4 * g + 4, :],
                        xT_r[:, 4 * g : 4 * g + 4, t0 : t0 + TH],
                    )
                xh_tiles[key] = xh
            return xh_tiles[key]

        pass_idx = [0]

        def proj_pass(b, h, half, w_sb, dest):
            xh = get_xh(b, half)
            n = pass_idx[0]
            pass_idx[0] += 1
            pp = accp.tile([128, TH], F32, tag=f"acc{n % 2}", name=f"pp{n}")
            for i in range(KC):
                for s in range(NSEG):
                    nc.tensor.matmul(
                        pp[:, s * 512 : (s + 1) * 512],
                        lhsT=w_sb[:, i, h * D : (h + 1) * D],
                        rhs=xh[:, i, s * 512 : (s + 1) * 512],
                        start=(i == 0),
                        stop=(i == KC - 1),
                    )
            dst = dest[:, h, half * TH : (half + 1) * TH]
            if n % 2 == 0:
                nc.vector.tensor_copy(dst, pp[:, :])
            else:
                nc.scalar.copy(dst, pp[:, :])

        def proj_quarter(b, h, half, seg, w_sb, dest):
            xh = get_xh(b, half)
            n = pass_idx[0]
            pass_idx[0] += 1
            pp = accp.tile([128, 512], F32, tag=f"acc{n % 2}", name=f"ppq{n}")
            for i in range(KC):
                nc.tensor.matmul(
                    pp[:, :],
                    lhsT=w_sb[:, i, h * D : (h + 1) * D],
                    rhs=xh[:, i, seg * 512 : (seg + 1) * 512],
                    start=(i == 0),
                    stop=(i == KC - 1),
                )
            dst = dest[:, h, half * TH + seg * 512 : half * TH + (seg + 1) * 512]
            if n % 2 == 0:
                nc.vector.tensor_copy(dst, pp[:, :])
            else:
                nc.scalar.copy(dst, pp[:, :])

        def score_chunk(b, h, qc):
            stage = stp.tile([128, S], F32, tag="stage", name=f"stage_{b}_{h}_{qc}")
            lq = qT[b][:, h, qc * 128 : (qc + 1) * 128]
            for half in range(2):
                ps = scp.tile([128, 1024], F32, tag="sc", name=f"sc_{b}_{h}_{qc}_{half}")
                for s2 in range(2):
                    nc.tensor.matmul(
                        ps[:, s2 * 512 : (s2 + 1) * 512],
                        lhsT=lq,
                        rhs=kT[b][:, h, half * 1024 + s2 * 512 : half * 1024 + (s2 + 1) * 512],
                        start=True,
                        stop=True,
                    )
                dst = stage[:, half * 1024 : (half + 1) * 1024]
                if (qc + half) % 2 == 0:
                    nc.vector.tensor_copy(dst, ps[:, :])
                else:
                    nc.scalar.copy(dst, ps[:, :])
            nc.scalar.dma_start(out_r[b, qc][:, h * S : (h + 1) * S], stage[:, :])

        # Per (batch, head): project k then q for both token halves; the
        # previous head's score chunks are woven between projection passes
        # so the PE stream alternates (keeps HAM warm, keeps write-DMA fed).
        from collections import deque

        pending = deque()
        for b in range(B):
            for h in range(HL):
                last = (b == B - 1) and (h == HL - 1)
                if not last:
                    for half in range(2):
                        proj_pass(b, h, half, wk_sb, kT[b])
                        for _ in range(min(2, len(pending))):
                            score_chunk(*pending.popleft())
                        proj_pass(b, h, half, wq_sb, qT[b])
                        for _ in range(min(2, len(pending))):
                            score_chunk(*pending.popleft())
                    pending.extend((b, h, qc) for qc in range(S // 128))
                else:
                    # final head: quarter-granular q-projections with its own
                    # score chunks woven immediately behind each quarter, so
                    # the kernel tail never runs scores alone
                    for half in range(2):
                        proj_pass(b, h, half, wk_sb, kT[b])
                        for _ in range(min(2, len(pending))):
                            score_chunk(*pending.popleft())
                    for half in range(2):
                        for seg in range(2):
                            proj_quarter(b, h, half, seg, wq_sb, qT[b])
                            for _ in range(min(2, len(pending))):
                                score_chunk(*pending.popleft())
                            for qc in range(4 * (2 * half + seg), 4 * (2 * half + seg) + 4):
                                score_chunk(b, h, qc)
        while pending:
            score_chunk(*pending.popleft())


def _dedup_ldweights(nc):
    """Drop LDWEIGHTS that reload the identical stationary operand.

    tile_legalize splits every matmul into LDWEIGHTS+MATMUL; the PE array
    keeps the stationary operand across matmuls, so a reload of the same
    AP with no intervening PE weight change is dead. Only sync-free
    instances are removed.
    """
    removed = 0
    for f in nc.m.functions:
        for bb in f.blocks:
            keep = []
            key = None
            for inst in bb.instructions:
                tn = type(inst).__name__
                if getattr(inst, "engine", None) == mybir.EngineType.PE:
                    if tn == "InstLdweights":
                        k = repr(inst.ins[0])
                        si = inst.sync_info
                        clean = not si or (not si.on_wait and not si.on_update)
                        if k == key and clean:
                            removed += 1
                            continue
                        key = k
                    elif tn not in ("InstMatmult", "InstNoOp", "InstEventSemaphore"):
                        key = None
                keep.append(inst)
            bb.instructions[:] = keep
    return removed


_NC = None


def _get_nc():
    global _NC
    if _NC is None:
        nc = bacc.Bacc("TRN2", debug=False)
        xT = nc.dram_tensor("xT", [IN, TOK], BF16, kind="ExternalInput")
        wqT = nc.dram_tensor("wqT", [IN, HD], BF16, kind="ExternalInput")
        wkT = nc.dram_tensor("wkT", [IN, HD], BF16, kind="ExternalInput")
        out = nc.dram_tensor("scores", [B, S, HL, S], F32, kind="ExternalOutput")
        with tile.TileContext(nc) as tc:
            _kernel_body(tc, xT, wqT, wkT, out)
        nc.compile()
        _dedup_ldweights(nc)
        _NC = nc
    return _NC


def _prepare(x, Wq, Wk):
    bf = ml_dtypes.bfloat16
    xT = np.ascontiguousarray(x.reshape(TOK, IN).T).astype(bf)
    in_maps = []
    for c in range(NCORES):
        rows = slice(c * HD, (c + 1) * HD)
        in_maps.append({
            "xT": xT,
            "wqT": np.ascontiguousarray((Wq[rows, :] * SCALE).T).astype(bf),
            "wkT": np.ascontiguousarray(Wk[rows, :].T).astype(bf),
        })
    return in_maps


def _run(x, Wq, Wk, trace=False):
    in_maps = _prepare(x, Wq, Wk)
    res = run_bass_kernel_spmd(_get_nc(), in_maps, core_ids=list(range(NCORES)), trace=trace)
    attn_weights = np.concatenate([res.results[c]["scores"] for c in range(NCORES)], axis=2)
    attn_output = np.zeros((B, S, HID), dtype=np.float32)
    return (attn_output, attn_weights), res


def kernel(x, Wq, Wk, Wv):
    x = np.asarray(x, dtype=np.float32)
    Wq = np.asarray(Wq, dtype=np.float32)
    Wk = np.asarray(Wk, dtype=np.float32)
    out, _ = _run(x, Wq, Wk, trace=False)
    return out


def kernel_traced(x, Wq, Wk, Wv):
    out, res = _run(
        np.asarray(x, dtype=np.float32),
        np.asarray(Wq, dtype=np.float32),
        np.asarray(Wk, dtype=np.float32),
        trace=True,
    )
    return out, res

